# revision 20
# baseline (speedup 1.0000x reference)
"""Causal self-attention (B=4, T=1024, C=2048, H=16, rotary) on 8 trn2 cores.

Sharding: core c = 2*b + g handles batch b, head-group g (heads 8g..8g+7).
 - QKV projection computed in transposed layout: Q^T/K^T = [d_channels, T],
   V in natural [T, d_channels] layout (for the att@V contraction).
 - RoPE via host-precomputed full-height cos/sin tables; the rotate-half
   partition swap runs on the PE as a permutation matmul.
 - Scores computed transposed S^T = K_tile^T . Q -> [k, q]; softmax without
   max-subtraction (logits are ~N(0,1); exp can't overflow); causal masking
   folded into the scores PSUM group as an additive -1e5 mask via an
   identity matmul (exp then lands exactly 0 for masked slots).
 - Softmax denominators for all 8 heads of a q-chunk accumulate into one
   [8, 512] PSUM tile via indicator-column matmuls; ONE reciprocal per
   q-chunk (vs 16 slow single-lane reciprocals), broadcast across
   partitions with a K=1 ones matmul on the PE (no DRAM bounce).
 - att@V accumulated in PSUM over k-tiles -> y^T [d, q]; copied to SBUF
   unnormalized, then scaled in-place by the broadcast reciprocal.
 - AllGather (pairs sharing a batch) split into 4 per-head-pair chunks
   fired progressively as heads finish, overlapping the remaining
   attention; c_proj consumes gathered chunks in landing order (w_proj
   rows pre-permuted on the host to match, identically for both cores of
   a pair, so the program stays SPMD).
All matmuls in bf16 (fp32 PSUM accumulation).
"""

import math

import numpy as np
import ml_dtypes

BF16 = ml_dtypes.bfloat16

B, T, C = 4, 1024, 2048
H = 16  # total heads
D = C // H  # 128 head dim
HG = 8  # heads per group (per core)
N_CORES = 8
ROPE_BASE = 10000.0

# AllGather chunking: attention runs pair-outer (both q-chunks of a head
# pair back to back) so each pair's AllGather fires as early as possible
# and overlaps the remaining attention.
AG_PAIRS = ((0, 1), (2, 3), (4, 5), (6, 7))

TUNE = {
    "chunk_order": (0, 2, 4, 1, 3, 5),
    "ps_bufs": 2,
    "ps_s_bufs": 2,
    "ps_y_bufs": 2,
    "p_sb_bufs": 6,
}

_PROGRAM_CACHE = {}


def _build_program(num_devices=N_CORES, collective=True, reps=1):
    import concourse.mybir as mybir
    import concourse.tile as tile
    from concourse import bacc
    from concourse.bass import ts

    f32 = mybir.dt.float32
    bf16 = mybir.dt.bfloat16
    AF = mybir.ActivationFunctionType

    nc = bacc.Bacc(trn_type="TRN2", num_devices=num_devices, debug=False)

    # ---- per-core I/O ----
    xT = nc.dram_tensor("xT", [C, T], bf16, kind="ExternalInput")  # x[b].T
    wqkv = nc.dram_tensor("wqkv", [C, 3 * HG * D], bf16, kind="ExternalInput")
    bqk = nc.dram_tensor("bqk", [128, 16], f32, kind="ExternalInput")
    bv = nc.dram_tensor("bv", [1, HG * D], f32, kind="ExternalInput")
    # full-height rope tables: cos2 = [cos; cos], sin2 = [-sin; sin]
    cosT = nc.dram_tensor("cosT", [D, T], bf16, kind="ExternalInput")
    sinT = nc.dram_tensor("sinT", [D, T], bf16, kind="ExternalInput")
    # additive causal mask blocks: 0 allowed / -1e5 masked, [k, r, q]
    maskT = nc.dram_tensor("maskT", [128, 4, 512], bf16, kind="ExternalInput")
    # half-swap permutation: perm[j2, j] = 1 iff j2 == (j + 64) % 128
    perm = nc.dram_tensor("perm", [128, 128], bf16, kind="ExternalInput")
    ident = nc.dram_tensor("ident", [128, 128], bf16, kind="ExternalInput")
    # indicator columns: ind2[k, 2*j + j'] = 1 iff j' == j
    ind2 = nc.dram_tensor("ind2", [128, 4], bf16, kind="ExternalInput")
    # row selectors: sel2[k, j*128 + m] = 1 iff k == j
    sel2 = nc.dram_tensor("sel2", [2, 2 * 128], bf16, kind="ExternalInput")
    wproj = nc.dram_tensor("wproj", [C, C // 2], bf16, kind="ExternalInput")
    bproj = nc.dram_tensor("bproj", [1, C // 2], f32, kind="ExternalInput")
    out = nc.dram_tensor("out", [T, C // 2], f32, kind="ExternalOutput")

    xT_r = xT.ap().rearrange("(ct p) t -> p ct t", p=128)  # [128, 16, 1024]
    wqkv_r = wqkv.ap().rearrange("(ct p) j -> p ct j", p=128)  # [128, 16, 3072]
    wproj_r = wproj.ap().rearrange("(jt p) c -> p jt c", p=128)  # [128, 16, 1024]

    scale = 1.0 / math.sqrt(D)

    with tile.TileContext(nc) as tc:
        with (
            tc.tile_pool(name="const", bufs=1) as const,
            tc.tile_pool(name="persist", bufs=1) as persist,
            tc.tile_pool(name="ps", bufs=TUNE["ps_bufs"], space="PSUM") as pspool,
            tc.tile_pool(
                name="ps_s", bufs=TUNE["ps_s_bufs"], space="PSUM"
            ) as ps_s_pool,
            tc.tile_pool(
                name="ps_y", bufs=TUNE["ps_y_bufs"], space="PSUM"
            ) as ps_y_pool,
            tc.tile_pool(name="ps_d", bufs=1, space="PSUM") as ps_d_pool,
            tc.tile_pool(name="work", bufs=4) as work,
            tc.tile_pool(name="dram", bufs=1, space="DRAM") as drampool,
        ):
            # ---- constants ----
            cos_sb = const.tile([128, T], bf16)
            nc.sync.dma_start(out=cos_sb, in_=cosT.ap())
            sin_sb = const.tile([128, T], bf16)
            nc.sync.dma_start(out=sin_sb, in_=sinT.ap())
            mask_sb = const.tile([128, 4, 512], bf16)
            nc.sync.dma_start(out=mask_sb, in_=maskT.ap())
            perm_sb = const.tile([128, 128], bf16)
            nc.sync.dma_start(out=perm_sb, in_=perm.ap())
            ident_sb = const.tile([128, 128], bf16)
            nc.sync.dma_start(out=ident_sb, in_=ident.ap())
            ind2_sb = const.tile([128, 4], bf16)
            nc.sync.dma_start(out=ind2_sb, in_=ind2.ap())
            sel2_sb = const.tile([2, 2 * 128], bf16)
            nc.sync.dma_start(out=sel2_sb, in_=sel2.ap())
            bqk_sb = const.tile([128, 16], f32)
            nc.sync.dma_start(out=bqk_sb, in_=bqk.ap())
            bv_bc = const.tile([128, HG * D], f32)
            nc.sync.dma_start(out=bv_bc, in_=bv.ap().to_broadcast([128, HG * D]))
            bp_bc = const.tile([128, C // 2], f32)
            nc.sync.dma_start(out=bp_bc, in_=bproj.ap().to_broadcast([128, C // 2]))

            # ---- persistent activations (reused across reps) ----
            qf = persist.tile([128, HG, T], bf16)  # [d, h, t] rotated Q^T
            kf = persist.tile([128, HG, T], bf16)  # [d, h, t] rotated K^T
            v_all = persist.tile([128, 8, HG * D], bf16)  # [t_in, tt, j]
            yT = persist.tile([128, HG, T], bf16)  # [d, h, t] normalized att out

            for rep in range(reps):
                _emit_once(
                    nc, tc, mybir, ts, f32, bf16, AF, scale, collective, rep,
                    xT_r, wqkv_r, wproj_r, out,
                    cos_sb, sin_sb, mask_sb, perm_sb, ident_sb, ind2_sb,
                    sel2_sb, bqk_sb, bv_bc, bp_bc,
                    qf, kf, v_all, yT,
                    pspool, ps_s_pool, ps_y_pool, ps_d_pool, work, drampool,
                )

    nc.finalize()
    return nc


def _emit_once(
    nc, tc, mybir, ts, f32, bf16, AF, scale, collective, rep,
    xT_r, wqkv_r, wproj_r, out,
    cos_sb, sin_sb, mask_sb, perm_sb, ident_sb, ind2_sb,
    sel2_sb, bqk_sb, bv_bc, bp_bc,
    qf, kf, v_all, yT,
    pspool, ps_s_pool, ps_y_pool, ps_d_pool, work, drampool,
):
    # =========== Phase A: QKV projection (+bias, +RoPE) ===========
    with (
        tc.tile_pool(name=f"xpool{rep}", bufs=1) as xpool,
        tc.tile_pool(name=f"wpool{rep}", bufs=2) as wpool,
    ):
        xs = xpool.tile([128, 16, T], bf16, name="xs")
        # first chunk's weights interleaved with x so the first matmul
        # group's inputs land as early as possible
        first_chunk = TUNE["chunk_order"][0]
        wt_first = wpool.tile([128, 16, 512], bf16, tag="wt", name="wt")
        for ct in range(16):
            nc.sync.dma_start(out=xs[:, ct, :], in_=xT_r[:, ct, :])
            nc.sync.dma_start(
                out=wt_first[:, ct, :],
                in_=wqkv_r[:, ct, first_chunk * 512 : (first_chunk + 1) * 512],
            )

        # order q0,k0,v0 first so heads 0-3 complete early
        for ci, chunk in enumerate(TUNE["chunk_order"]):
            if ci == 0:
                wt = wt_first
            else:
                wt = wpool.tile([128, 16, 512], bf16, tag="wt", name="wt")
                for ct in range(16):
                    nc.sync.dma_start(
                        out=wt[:, ct, :],
                        in_=wqkv_r[:, ct, chunk * 512 : (chunk + 1) * 512],
                    )
            if chunk < 4:  # Q or K, output transposed [j, t]
                for jj in range(4):
                    jt = chunk * 4 + jj  # 0..15 (q: 0-7, k: 8-15)
                    h = jt % 8
                    dest_all = qf if jt < 8 else kf
                    for th in range(2):  # t halves of 512
                        ps = pspool.tile([128, 512], f32, tag="ps", name="ps")
                        for ct in range(16):
                            nc.tensor.matmul(
                                ps,
                                lhsT=wt[:, ct, jj * 128 : (jj + 1) * 128],
                                rhs=xs[:, ct, ts(th, 512)],
                                start=(ct == 0),
                                stop=(ct == 15),
                            )
                        # bias add + PSUM->SBUF copy on the (idle) ACT engine
                        raw = work.tile([128, 512], bf16, tag="raw", name="raw")
                        nc.scalar.activation(
                            raw, ps, AF.Identity, bias=bqk_sb[:, jt : jt + 1]
                        )
                        # RoPE: out = raw*cos2 + swap_halves(raw)*sin2
                        # half-swap on PE via permutation matmul (DVE
                        # can't move data across partitions)
                        dest = dest_all[:, h, ts(th, 512)]
                        ps_swp = ps_s_pool.tile(
                            [128, 512], f32, tag="ps_sc", name="ps_swp"
                        )
                        nc.tensor.matmul(
                            ps_swp, lhsT=perm_sb, rhs=raw, start=True, stop=True
                        )
                        rtmp = work.tile([128, 512], bf16, tag="rtmp", name="rtmp")
                        nc.vector.tensor_mul(rtmp, ps_swp, sin_sb[:, ts(th, 512)])
                        nc.vector.tensor_mul(dest, raw, cos_sb[:, ts(th, 512)])
                        nc.vector.tensor_add(dest, dest, rtmp)
            else:  # V, natural layout [t, j]
                jc = chunk - 4  # 0 or 1
                for tt in range(8):
                    ps = pspool.tile([128, 512], f32, tag="ps", name="ps")
                    for ct in range(16):
                        nc.tensor.matmul(
                            ps,
                            lhsT=xs[:, ct, ts(tt, 128)],
                            rhs=wt[:, ct, :],
                            start=(ct == 0),
                            stop=(ct == 15),
                        )
                    nc.vector.tensor_add(
                        v_all[:, tt, jc * 512 : (jc + 1) * 512],
                        ps,
                        bv_bc[:, jc * 512 : (jc + 1) * 512],
                    )

    # proj weights: load now so the DMA overlaps phase B
    projpool = tc.tile_pool(name=f"proj{rep}", bufs=1)
    proj = projpool.__enter__()
    try:
        wp = proj.tile([128, 16, C // 2], bf16, name="wp")
        nc.sync.dma_start(out=wp, in_=wproj_r)
        ygs = proj.tile([128, 16, T], bf16, name="ygs")

        ybounce = drampool.tile([HG * D, T], bf16, name="ybounce")
        yb_r = ybounce.rearrange("(h p) t -> p h t", p=128)
        ygth = [
            drampool.tile([512, T], bf16, name=f"ygth{c}", tag=f"ygth{c}")
            for c in range(4)
        ]

        # =========== Phase B: attention, head-pair outer ===========
        for c, pair in enumerate(AG_PAIRS):
            for qc in range(2):
                n_kt = 4 * (qc + 1)  # causal: valid k tiles
                denom_ps = ps_d_pool.tile(
                    [2, 512], f32, tag="denom", name="denom", bufs=2
                )
                for j, h in enumerate(pair):
                    ps_y = ps_y_pool.tile(
                        [128, 512], f32, tag="ps_y", name="ps_y"
                    )
                    for kt in range(n_kt):
                        kt_rel = kt - 4 * qc
                        diag = 0 <= kt_rel < 4  # straddles the diagonal
                        ps_sc = ps_s_pool.tile(
                            [128, 512], f32, tag="ps_sc", name="ps_sc"
                        )
                        nc.tensor.matmul(
                            ps_sc,
                            lhsT=kf[:, h, ts(kt, 128)],
                            rhs=qf[:, h, ts(qc, 512)],
                            start=True,
                            stop=not diag,
                        )
                        if diag:  # additive -1e5 mask into the PSUM group
                            nc.tensor.matmul(
                                ps_sc,
                                lhsT=ident_sb,
                                rhs=mask_sb[:, kt_rel, :],
                                start=False,
                                stop=True,
                            )
                        p_sb = work.tile(
                            [128, 512], bf16, tag="p_sb", name="p_sb",
                            bufs=TUNE["p_sb_bufs"],
                        )
                        nc.scalar.activation(p_sb, ps_sc, AF.Exp, scale=scale)
                        # denominator row j accumulates via indicator column
                        nc.tensor.matmul(
                            denom_ps,
                            lhsT=ind2_sb[:, 2 * j : 2 * j + 2],
                            rhs=p_sb,
                            start=(j == 0 and kt == 0),
                            stop=(j == 1 and kt == n_kt - 1),
                        )
                        nc.tensor.matmul(
                            ps_y,
                            lhsT=v_all[:, kt, ts(h, 128)],
                            rhs=p_sb,
                            start=(kt == 0),
                            stop=(kt == n_kt - 1),
                        )
                    # park unnormalized y^T in SBUF; scaled in-place below
                    nc.vector.tensor_copy(yT[:, h, ts(qc, 512)], ps_y)

                # ---- normalization: per-pair reciprocal on the DVE (Ln on
                # ACT would thrash the activation table set the exps use)
                recip_sb = work.tile(
                    [2, 512], bf16, tag="recip_sb", name="recip_sb", bufs=2
                )
                with nc.allow_low_precision(
                    reason="bf16 softmax reciprocal; matches overall bf16 noise"
                ):
                    nc.vector.reciprocal(recip_sb, denom_ps)
                for j, h in enumerate(pair):
                    # select + broadcast recip row j across partitions via
                    # a K=2 matmul with a row-selector stationary operand
                    rps = ps_y_pool.tile(
                        [128, 512], f32, tag="ps_y", name="rps"
                    )
                    nc.tensor.matmul(
                        rps,
                        lhsT=sel2_sb[:, j * 128 : (j + 1) * 128],
                        rhs=recip_sb,
                        start=True,
                        stop=True,
                    )
                    nc.vector.tensor_mul(
                        yT[:, h, ts(qc, 512)], yT[:, h, ts(qc, 512)], rps
                    )

            # ---- pair fully done: ship it and start its gather chunk ----
            for h in pair:
                nc.sync.dma_start(out=yb_r[:, h, :], in_=yT[:, h, :])
            rows = slice(pair[0] * 128, (pair[1] + 1) * 128)
            if collective:
                nc.gpsimd.collective_compute(
                    "AllGather",
                    mybir.AluOpType.bypass,
                    replica_groups=[[0, 1], [2, 3], [4, 5], [6, 7]],
                    ins=[ybounce[rows, :].opt()],
                    outs=[ygth[c][:].opt()],
                )
            else:  # timeline-sim variant: fake the gather with local copies
                nc.sync.dma_start(out=ygth[c][0:256, :], in_=ybounce[rows, :])
                nc.sync.dma_start(out=ygth[c][256:512, :], in_=ybounce[rows, :])
            nc.sync.dma_start(
                out=ygs[:, 4 * c : 4 * c + 4, :],
                in_=ygth[c].rearrange("(jt p) t -> p jt t", p=128),
            )

        # =========== Phase C: c_proj over gathered chunks ===========
        # jt order == chunk landing order; w_proj rows host-permuted to match
        for tt in range(8):
            for cc in range(2):  # output col chunks of 512
                ps = pspool.tile([128, 512], f32, tag="ps", name="ps_proj")
                for jt in range(16):
                    nc.tensor.matmul(
                        ps,
                        lhsT=ygs[:, jt, ts(tt, 128)],
                        rhs=wp[:, jt, ts(cc, 512)],
                        start=(jt == 0),
                        stop=(jt == 15),
                    )
                o_sb = work.tile([128, 512], f32, tag="o_sb", name="o_sb")
                nc.vector.tensor_add(o_sb, ps, bp_bc[:, ts(cc, 512)])
                nc.sync.dma_start(
                    out=out.ap()[ts(tt, 128), ts(cc, 512)], in_=o_sb
                )
    finally:
        projpool.__exit__(None, None, None)


def _host_inputs(x, w_attn, b_attn, w_proj, b_proj):
    """Build the 8 per-core input maps."""
    x = np.asarray(x, np.float32)
    w_attn = np.asarray(w_attn, np.float32)
    b_attn = np.asarray(b_attn, np.float32)
    w_proj = np.asarray(w_proj, np.float32)
    b_proj = np.asarray(b_proj, np.float32)

    # rope tables, transposed [d, t], full height with rotate-half signs folded:
    # out = x * cos2 + swap_halves(x) * sin2,  cos2=[cos;cos], sin2=[-sin;sin]
    inv_freq = 1.0 / (ROPE_BASE ** (np.arange(0, D, 2, dtype=np.float32) / D))
    freqs = np.arange(T, dtype=np.float32)[:, None] * inv_freq[None, :]  # [T, 64]
    c_ = np.ascontiguousarray(np.cos(freqs).T)  # [64, T]
    s_ = np.ascontiguousarray(np.sin(freqs).T)
    cosT = np.concatenate([c_, c_], axis=0).astype(BF16)  # [128, T]
    sinT = np.concatenate([-s_, s_], axis=0).astype(BF16)

    # additive causal mask blocks, transposed [k, q]: 0 allowed / -1e5 masked
    k_idx = np.arange(128)
    q_idx = np.arange(512)
    maskT = np.zeros((128, 4, 512), np.float32)
    for r in range(4):
        maskT[:, r, :] = np.where(
            (r * 128 + k_idx)[:, None] > q_idx[None, :], -1e5, 0.0
        )
    maskT = maskT.astype(BF16)

    permM = np.zeros((128, 128), np.float32)
    permM[(np.arange(128) + 64) % 128, np.arange(128)] = 1.0
    permM = permM.astype(BF16)

    identM = np.eye(128, dtype=np.float32).astype(BF16)

    ind2 = np.zeros((128, 4), np.float32)
    for j in range(2):
        ind2[:, 2 * j + j] = 1.0
    ind2 = ind2.astype(BF16)

    sel2 = np.zeros((2, 2 * 128), np.float32)
    for j in range(2):
        sel2[j, j * 128 : (j + 1) * 128] = 1.0
    sel2 = sel2.astype(BF16)

    # w_proj row permutation: rows grouped to match AllGather chunk
    # landing order [pair0: g0 heads, g1 heads; pair1: ...]; identical
    # for both cores of a batch pair (keeps the program SPMD).
    row_order = []
    for pr in AG_PAIRS:
        for g in range(2):
            for h in pr:
                base = g * 1024 + h * 128
                row_order.extend(range(base, base + 128))
    row_order = np.asarray(row_order)

    in_maps = []
    for c in range(N_CORES):
        b, g = divmod(c, 2)
        cs = slice(g * 1024, (g + 1) * 1024)
        wq = w_attn[:, 0:C][:, cs]
        wk = w_attn[:, C : 2 * C][:, cs]
        wv = w_attn[:, 2 * C : 3 * C][:, cs]
        bq = b_attn[0:C][cs]
        bk = b_attn[C : 2 * C][cs]
        bvv = b_attn[2 * C : 3 * C][cs]
        in_maps.append(
            {
                "xT": np.ascontiguousarray(x[b].T).astype(BF16),
                "wqkv": np.concatenate([wq, wk, wv], axis=1).astype(BF16),
                "bqk": np.ascontiguousarray(
                    np.concatenate([bq, bk]).reshape(16, 128).T
                ).astype(np.float32),
                "bv": bvv.reshape(1, 1024).astype(np.float32),
                "cosT": cosT,
                "sinT": sinT,
                "maskT": maskT,
                "perm": permM,
                "ident": identM,
                "ind2": ind2,
                "sel2": sel2,
                "wproj": w_proj[row_order][:, cs].astype(BF16),
                "bproj": b_proj[cs].reshape(1, 1024).astype(np.float32),
            }
        )
    return in_maps


def kernel(x, w_attn, b_attn, w_proj, b_proj, _trace=False):
    from concourse.bass_utils import run_bass_kernel_spmd

    if "nc" not in _PROGRAM_CACHE:
        _PROGRAM_CACHE["nc"] = _build_program()
    nc = _PROGRAM_CACHE["nc"]

    in_maps = _host_inputs(x, w_attn, b_attn, w_proj, b_proj)
    res = run_bass_kernel_spmd(
        nc, in_maps, core_ids=list(range(N_CORES)), trace=_trace
    )
    _PROGRAM_CACHE["last_results"] = res

    out = np.zeros((B, T, C), np.float32)
    for c in range(N_CORES):
        b, g = divmod(c, 2)
        out[b, :, g * 1024 : (g + 1) * 1024] = res.results[c]["out"]
    return out


# revision 26
# speedup vs baseline: 1.3798x; 1.3798x over previous
"""Causal self-attention (B=4, T=1024, C=2048, H=16, rotary) on 8 trn2 cores.

Sharding: core c = 2*b + g handles batch b, head-group g (heads 8g..8g+7).
 - QKV projection computed in transposed layout: Q^T/K^T = [d_channels, T],
   V in natural [T, d_channels] layout (for the att@V contraction).
 - RoPE via host-precomputed full-height cos/sin tables; the rotate-half
   partition swap runs on the PE as a permutation matmul.
 - Scores computed transposed S^T = K_tile^T . Q -> [k, q]; softmax without
   max-subtraction (logits are ~N(0,1); exp can't overflow); causal masking
   folded into the scores PSUM group as an additive -1e5 mask via an
   identity matmul (exp then lands exactly 0 for masked slots).
 - Softmax denominators for all 8 heads of a q-chunk accumulate into one
   [8, 512] PSUM tile via indicator-column matmuls; ONE reciprocal per
   q-chunk (vs 16 slow single-lane reciprocals), broadcast across
   partitions with a K=1 ones matmul on the PE (no DRAM bounce).
 - att@V accumulated in PSUM over k-tiles -> y^T [d, q]; copied to SBUF
   unnormalized, then scaled in-place by the broadcast reciprocal.
 - AllGather (pairs sharing a batch) split into 4 per-head-pair chunks
   fired progressively as heads finish, overlapping the remaining
   attention; c_proj consumes gathered chunks in landing order (w_proj
   rows pre-permuted on the host to match, identically for both cores of
   a pair, so the program stays SPMD).
All matmuls in bf16 (fp32 PSUM accumulation).
"""

import math

import numpy as np
import ml_dtypes

BF16 = ml_dtypes.bfloat16

B, T, C = 4, 1024, 2048
H = 16  # total heads
D = C // H  # 128 head dim
HG = 8  # heads per group (per core)
N_CORES = 8
ROPE_BASE = 10000.0

# AllGather chunking: attention runs pair-outer (both q-chunks of a head
# pair back to back) so each pair's AllGather fires as early as possible
# and overlaps the remaining attention.
AG_PAIRS = ((0, 1), (2, 3), (4, 5), (6, 7))

TUNE = {
    "chunk_order": (0, 2, 4, 1, 3, 5),
    "ps_bufs": 2,
    "ps_s_bufs": 2,
    "ps_y_bufs": 2,
    "p_sb_bufs": 6,
}

_PROGRAM_CACHE = {}


def _build_program(num_devices=N_CORES, collective=True, reps=1):
    import concourse.mybir as mybir
    import concourse.tile as tile
    from concourse import bacc
    from concourse.bass import ts

    f32 = mybir.dt.float32
    bf16 = mybir.dt.bfloat16
    AF = mybir.ActivationFunctionType

    nc = bacc.Bacc(trn_type="TRN2", num_devices=num_devices, debug=False)

    # ---- per-core I/O ----
    xT = nc.dram_tensor("xT", [C, T], bf16, kind="ExternalInput")  # x[b].T
    wqkv = nc.dram_tensor("wqkv", [C, 3 * HG * D], bf16, kind="ExternalInput")
    bqk = nc.dram_tensor("bqk", [128, 16], f32, kind="ExternalInput")
    bv = nc.dram_tensor("bv", [1, HG * D], f32, kind="ExternalInput")
    # full-height rope tables: cos2 = [cos; cos], sin2 = [-sin; sin]
    cosT = nc.dram_tensor("cosT", [D, T], bf16, kind="ExternalInput")
    sinT = nc.dram_tensor("sinT", [D, T], bf16, kind="ExternalInput")
    # additive causal mask blocks: 0 allowed / -1e5 masked, [k, r, q]
    maskT = nc.dram_tensor("maskT", [128, 4, 512], bf16, kind="ExternalInput")
    # half-swap permutation: perm[j2, j] = 1 iff j2 == (j + 64) % 128
    perm = nc.dram_tensor("perm", [128, 128], bf16, kind="ExternalInput")
    ident = nc.dram_tensor("ident", [128, 128], bf16, kind="ExternalInput")
    # indicator columns: ind2[k, 2*j + j'] = 1 iff j' == j
    ind2 = nc.dram_tensor("ind2", [128, 4], bf16, kind="ExternalInput")
    # row selectors: sel2[k, j*128 + m] = 1 iff k == j
    sel2 = nc.dram_tensor("sel2", [2, 2 * 128], bf16, kind="ExternalInput")
    wproj = nc.dram_tensor("wproj", [C, C // 2], bf16, kind="ExternalInput")
    bproj = nc.dram_tensor("bproj", [1, C // 2], f32, kind="ExternalInput")
    out = nc.dram_tensor("out", [T, C // 2], f32, kind="ExternalOutput")

    xT_r = xT.ap().rearrange("(ct p) t -> p ct t", p=128)  # [128, 16, 1024]
    wqkv_r = wqkv.ap().rearrange("(ct p) j -> p ct j", p=128)  # [128, 16, 3072]
    wproj_r = wproj.ap().rearrange("(jt p) c -> p jt c", p=128)  # [128, 16, 1024]

    scale = 1.0 / math.sqrt(D)

    with tile.TileContext(nc) as tc:
        with (
            tc.tile_pool(name="const", bufs=1) as const,
            tc.tile_pool(name="persist", bufs=1) as persist,
            tc.tile_pool(name="ps", bufs=TUNE["ps_bufs"], space="PSUM") as pspool,
            tc.tile_pool(
                name="ps_s", bufs=TUNE["ps_s_bufs"], space="PSUM"
            ) as ps_s_pool,
            tc.tile_pool(
                name="ps_y", bufs=TUNE["ps_y_bufs"], space="PSUM"
            ) as ps_y_pool,
            tc.tile_pool(name="ps_d", bufs=1, space="PSUM") as ps_d_pool,
            tc.tile_pool(name="work", bufs=4) as work,
            tc.tile_pool(name="dram", bufs=1, space="DRAM") as drampool,
        ):
            # ---- constants: the phase-A-critical ones (rope tables, qk
            # bias, perm) load immediately; the rest defer until the first
            # input tiles are in flight so the first matmuls start sooner
            perm_sb = const.tile([128, 128], bf16)
            nc.sync.dma_start(out=perm_sb, in_=perm.ap())
            bqk_sb = const.tile([128, 16], f32)
            nc.sync.dma_start(out=bqk_sb, in_=bqk.ap())
            cos_sb = const.tile([128, T], bf16)
            nc.sync.dma_start(out=cos_sb, in_=cosT.ap())
            sin_sb = const.tile([128, T], bf16)
            nc.sync.dma_start(out=sin_sb, in_=sinT.ap())
            mask_sb = const.tile([128, 4, 512], bf16)
            ident_sb = const.tile([128, 128], bf16)
            ind2_sb = const.tile([128, 4], bf16)
            sel2_sb = const.tile([2, 2 * 128], bf16)
            bv_bc = const.tile([128, HG * D], f32)
            bp_bc = const.tile([128, C // 2], f32)

            def emit_const_dmas():
                nc.sync.dma_start(out=mask_sb, in_=maskT.ap())
                nc.sync.dma_start(out=ident_sb, in_=ident.ap())
                nc.sync.dma_start(out=ind2_sb, in_=ind2.ap())
                nc.sync.dma_start(out=sel2_sb, in_=sel2.ap())
                nc.sync.dma_start(
                    out=bv_bc, in_=bv.ap().to_broadcast([128, HG * D])
                )
                nc.sync.dma_start(
                    out=bp_bc, in_=bproj.ap().to_broadcast([128, C // 2])
                )

            # ---- persistent activations (reused across reps) ----
            qf = persist.tile([128, HG, T], bf16)  # [d, h, t] rotated Q^T
            kf = persist.tile([128, HG, T], bf16)  # [d, h, t] rotated K^T
            v_all = persist.tile([128, 8, HG * D], bf16)  # [t_in, tt, j]
            yT = persist.tile([128, HG, T], bf16)  # [d, h, t] normalized att out

            for rep in range(reps):
                _emit_once(
                    nc, tc, mybir, ts, f32, bf16, AF, scale, collective, rep,
                    xT_r, wqkv_r, wproj_r, out,
                    cos_sb, sin_sb, mask_sb, perm_sb, ident_sb, ind2_sb,
                    sel2_sb, bqk_sb, bv_bc, bp_bc,
                    qf, kf, v_all, yT,
                    pspool, ps_s_pool, ps_y_pool, ps_d_pool, work, drampool,
                    emit_const_dmas if rep == 0 else None,
                )

    nc.finalize()
    return nc


def _emit_once(
    nc, tc, mybir, ts, f32, bf16, AF, scale, collective, rep,
    xT_r, wqkv_r, wproj_r, out,
    cos_sb, sin_sb, mask_sb, perm_sb, ident_sb, ind2_sb,
    sel2_sb, bqk_sb, bv_bc, bp_bc,
    qf, kf, v_all, yT,
    pspool, ps_s_pool, ps_y_pool, ps_d_pool, work, drampool,
    emit_const_dmas,
):
    # =========== Phase A: QKV projection (+bias, +RoPE) ===========
    with (
        tc.tile_pool(name=f"xpool{rep}", bufs=1) as xpool,
        tc.tile_pool(name=f"wpool{rep}", bufs=2) as wpool,
    ):
        xs = xpool.tile([128, 16, T], bf16, name="xs")
        # first chunk's weights interleaved with x (first t-half first) so
        # the first matmul group's inputs land as early as possible
        first_chunk = TUNE["chunk_order"][0]
        wt_first = wpool.tile([128, 16, 512], bf16, tag="wt", name="wt")
        for ct in range(16):
            nc.sync.dma_start(out=xs[:, ct, 0:512], in_=xT_r[:, ct, 0:512])
            nc.sync.dma_start(
                out=wt_first[:, ct, :],
                in_=wqkv_r[:, ct, first_chunk * 512 : (first_chunk + 1) * 512],
            )
        for ct in range(16):
            nc.sync.dma_start(out=xs[:, ct, 512:1024], in_=xT_r[:, ct, 512:1024])
        if emit_const_dmas is not None:
            emit_const_dmas()

        # order q0,k0,v0 first so heads 0-3 complete early
        for ci, chunk in enumerate(TUNE["chunk_order"]):
            if ci == 0:
                wt = wt_first
            else:
                wt = wpool.tile([128, 16, 512], bf16, tag="wt", name="wt")
                for ct in range(16):
                    nc.sync.dma_start(
                        out=wt[:, ct, :],
                        in_=wqkv_r[:, ct, chunk * 512 : (chunk + 1) * 512],
                    )
            if chunk < 4:  # Q or K, output transposed [j, t]
                for jj in range(4):
                    jt = chunk * 4 + jj  # 0..15 (q: 0-7, k: 8-15)
                    h = jt % 8
                    dest_all = qf if jt < 8 else kf
                    for th in range(2):  # t halves of 512
                        ps = pspool.tile([128, 512], f32, tag="ps", name="ps")
                        for ct in range(16):
                            nc.tensor.matmul(
                                ps,
                                lhsT=wt[:, ct, jj * 128 : (jj + 1) * 128],
                                rhs=xs[:, ct, ts(th, 512)],
                                start=(ct == 0),
                                stop=(ct == 15),
                            )
                        # bias add + PSUM->SBUF copy on the (idle) ACT engine
                        raw = work.tile([128, 512], bf16, tag="raw", name="raw")
                        nc.scalar.activation(
                            raw, ps, AF.Identity, bias=bqk_sb[:, jt : jt + 1]
                        )
                        # RoPE: out = raw*cos2 + swap_halves(raw)*sin2
                        # half-swap on PE via permutation matmul (DVE
                        # can't move data across partitions)
                        dest = dest_all[:, h, ts(th, 512)]
                        ps_swp = ps_s_pool.tile(
                            [128, 512], f32, tag="ps_sc", name="ps_swp"
                        )
                        nc.tensor.matmul(
                            ps_swp, lhsT=perm_sb, rhs=raw, start=True, stop=True
                        )
                        rtmp = work.tile([128, 512], bf16, tag="rtmp", name="rtmp")
                        nc.vector.tensor_mul(rtmp, ps_swp, sin_sb[:, ts(th, 512)])
                        nc.vector.tensor_mul(dest, raw, cos_sb[:, ts(th, 512)])
                        nc.vector.tensor_add(dest, dest, rtmp)
            else:  # V, natural layout [t, j]
                jc = chunk - 4  # 0 or 1
                for tt in range(8):
                    ps = pspool.tile([128, 512], f32, tag="ps", name="ps")
                    for ct in range(16):
                        nc.tensor.matmul(
                            ps,
                            lhsT=xs[:, ct, ts(tt, 128)],
                            rhs=wt[:, ct, :],
                            start=(ct == 0),
                            stop=(ct == 15),
                        )
                    nc.vector.tensor_add(
                        v_all[:, tt, jc * 512 : (jc + 1) * 512],
                        ps,
                        bv_bc[:, jc * 512 : (jc + 1) * 512],
                    )

    # proj weights: load now so the DMA overlaps phase B
    projpool = tc.tile_pool(name=f"proj{rep}", bufs=1)
    proj = projpool.__enter__()
    try:
        wp = proj.tile([128, 16, C // 2], bf16, name="wp")
        nc.sync.dma_start(out=wp, in_=wproj_r)
        ygs = proj.tile([128, 16, T], bf16, name="ygs")

        ybounce = drampool.tile([HG * D, T], bf16, name="ybounce")
        yb_r = ybounce.rearrange("(h p) t -> p h t", p=128)
        ygth = [
            drampool.tile([512, T], bf16, name=f"ygth{c}", tag=f"ygth{c}")
            for c in range(4)
        ]

        # =========== Phase B: attention, head-pair outer ===========
        # Normalization/bounce/AllGather for a finished (pair, qc) is
        # emitted a few k-tiles into the NEXT (pair, qc)'s stream so the
        # DVE reciprocal never head-of-line-blocks the PE queue.
        pending = []  # [(pair_idx, pair, qc, recip_sb)]

        def flush_pending():
            while pending:
                pc, ppair, pqc, prec = pending.pop(0)
                for j, h in enumerate(ppair):
                    # select + broadcast recip row j across partitions via
                    # a K=2 matmul with a row-selector stationary operand
                    rps = ps_s_pool.tile(
                        [128, 512], f32, tag="ps_sc", name="rps"
                    )
                    nc.tensor.matmul(
                        rps,
                        lhsT=sel2_sb[:, j * 128 : (j + 1) * 128],
                        rhs=prec,
                        start=True,
                        stop=True,
                    )
                    nc.vector.tensor_mul(
                        yT[:, h, ts(pqc, 512)], yT[:, h, ts(pqc, 512)], rps
                    )
                if pqc == 1:  # pair fully normalized: ship + gather chunk
                    for h in ppair:
                        nc.sync.dma_start(out=yb_r[:, h, :], in_=yT[:, h, :])
                    rows = slice(ppair[0] * 128, (ppair[1] + 1) * 128)
                    if collective:
                        nc.gpsimd.collective_compute(
                            "AllGather",
                            mybir.AluOpType.bypass,
                            replica_groups=[[0, 1], [2, 3], [4, 5], [6, 7]],
                            ins=[ybounce[rows, :].opt()],
                            outs=[ygth[pc][:].opt()],
                        )
                    else:  # timeline-sim variant: fake with local copies
                        nc.sync.dma_start(
                            out=ygth[pc][0:256, :], in_=ybounce[rows, :]
                        )
                        nc.sync.dma_start(
                            out=ygth[pc][256:512, :], in_=ybounce[rows, :]
                        )
                    nc.sync.dma_start(
                        out=ygs[:, 4 * pc : 4 * pc + 4, :],
                        in_=ygth[pc].rearrange("(jt p) t -> p jt t", p=128),
                    )

        for c, pair in enumerate(AG_PAIRS):
            for qc in range(2):
                n_kt = 4 * (qc + 1)  # causal: valid k tiles
                denom_ps = ps_d_pool.tile(
                    [2, 512], f32, tag="denom", name="denom", bufs=2
                )
                for j, h in enumerate(pair):
                    ps_y = ps_y_pool.tile(
                        [128, 512], f32, tag="ps_y", name="ps_y"
                    )
                    for kt in range(n_kt):
                        kt_rel = kt - 4 * qc
                        diag = 0 <= kt_rel < 4  # straddles the diagonal
                        ps_sc = ps_s_pool.tile(
                            [128, 512], f32, tag="ps_sc", name="ps_sc"
                        )
                        nc.tensor.matmul(
                            ps_sc,
                            lhsT=kf[:, h, ts(kt, 128)],
                            rhs=qf[:, h, ts(qc, 512)],
                            start=True,
                            stop=not diag,
                        )
                        if diag:  # additive -1e5 mask into the PSUM group
                            nc.tensor.matmul(
                                ps_sc,
                                lhsT=ident_sb,
                                rhs=mask_sb[:, kt_rel, :],
                                start=False,
                                stop=True,
                            )
                        p_sb = work.tile(
                            [128, 512], bf16, tag="p_sb", name="p_sb",
                            bufs=TUNE["p_sb_bufs"],
                        )
                        nc.scalar.activation(p_sb, ps_sc, AF.Exp, scale=scale)
                        # denominator row j accumulates via indicator column
                        nc.tensor.matmul(
                            denom_ps,
                            lhsT=ind2_sb[:, 2 * j : 2 * j + 2],
                            rhs=p_sb,
                            start=(j == 0 and kt == 0),
                            stop=(j == 1 and kt == n_kt - 1),
                        )
                        nc.tensor.matmul(
                            ps_y,
                            lhsT=v_all[:, kt, ts(h, 128)],
                            rhs=p_sb,
                            start=(kt == 0),
                            stop=(kt == n_kt - 1),
                        )
                        if j == 0 and kt == 2:
                            flush_pending()  # prev (pair, qc): recip done
                    # park unnormalized y^T in SBUF; scaled in-place later
                    nc.vector.tensor_copy(yT[:, h, ts(qc, 512)], ps_y)

                # per-pair reciprocal on the DVE (Ln on ACT would thrash
                # the activation table set the exps use)
                recip_sb = work.tile(
                    [2, 512], bf16, tag="recip_sb", name="recip_sb", bufs=2
                )
                with nc.allow_low_precision(
                    reason="bf16 softmax recip; matches overall bf16 noise"
                ):
                    nc.vector.reciprocal(recip_sb, denom_ps)
                pending.append((c, pair, qc, recip_sb))

        flush_pending()  # last pair's normalize + gather

        # =========== Phase C: c_proj over gathered chunks ===========
        # jt order == chunk landing order; w_proj rows host-permuted to
        # match. PSUM groups spread across all four pool tags so up to 8
        # output tiles pre-accumulate their first 12 jts while the last
        # gather chunk is still in flight.
        cpools = [
            (pspool, "ps", None),
            (ps_s_pool, "ps_sc", None),
            (ps_y_pool, "ps_y", None),
            (ps_d_pool, "denom", 2),
        ]
        for i, (tt, cc) in enumerate(
            (tt, cc) for tt in range(8) for cc in range(2)
        ):
            pool, tag, pbufs = cpools[i % 4]
            kw = {"bufs": pbufs} if pbufs is not None else {}
            ps = pool.tile([128, 512], f32, tag=tag, name="ps_proj", **kw)
            for jt in range(16):
                nc.tensor.matmul(
                    ps,
                    lhsT=ygs[:, jt, ts(tt, 128)],
                    rhs=wp[:, jt, ts(cc, 512)],
                    start=(jt == 0),
                    stop=(jt == 15),
                )
            o_sb = work.tile([128, 512], f32, tag="o_sb", name="o_sb")
            nc.vector.tensor_add(o_sb, ps, bp_bc[:, ts(cc, 512)])
            nc.sync.dma_start(
                out=out.ap()[ts(tt, 128), ts(cc, 512)], in_=o_sb
            )
    finally:
        projpool.__exit__(None, None, None)


def _host_inputs(x, w_attn, b_attn, w_proj, b_proj):
    """Build the 8 per-core input maps."""
    x = np.asarray(x, np.float32)
    w_attn = np.asarray(w_attn, np.float32)
    b_attn = np.asarray(b_attn, np.float32)
    w_proj = np.asarray(w_proj, np.float32)
    b_proj = np.asarray(b_proj, np.float32)

    # rope tables, transposed [d, t], full height with rotate-half signs folded:
    # out = x * cos2 + swap_halves(x) * sin2,  cos2=[cos;cos], sin2=[-sin;sin]
    inv_freq = 1.0 / (ROPE_BASE ** (np.arange(0, D, 2, dtype=np.float32) / D))
    freqs = np.arange(T, dtype=np.float32)[:, None] * inv_freq[None, :]  # [T, 64]
    c_ = np.ascontiguousarray(np.cos(freqs).T)  # [64, T]
    s_ = np.ascontiguousarray(np.sin(freqs).T)
    cosT = np.concatenate([c_, c_], axis=0).astype(BF16)  # [128, T]
    sinT = np.concatenate([-s_, s_], axis=0).astype(BF16)

    # additive causal mask blocks, transposed [k, q]: 0 allowed / -1e5 masked
    k_idx = np.arange(128)
    q_idx = np.arange(512)
    maskT = np.zeros((128, 4, 512), np.float32)
    for r in range(4):
        maskT[:, r, :] = np.where(
            (r * 128 + k_idx)[:, None] > q_idx[None, :], -1e5, 0.0
        )
    maskT = maskT.astype(BF16)

    permM = np.zeros((128, 128), np.float32)
    permM[(np.arange(128) + 64) % 128, np.arange(128)] = 1.0
    permM = permM.astype(BF16)

    identM = np.eye(128, dtype=np.float32).astype(BF16)

    ind2 = np.zeros((128, 4), np.float32)
    for j in range(2):
        ind2[:, 2 * j + j] = 1.0
    ind2 = ind2.astype(BF16)

    sel2 = np.zeros((2, 2 * 128), np.float32)
    for j in range(2):
        sel2[j, j * 128 : (j + 1) * 128] = 1.0
    sel2 = sel2.astype(BF16)

    # w_proj row permutation: rows grouped to match AllGather chunk
    # landing order [pair0: g0 heads, g1 heads; pair1: ...]; identical
    # for both cores of a batch pair (keeps the program SPMD).
    row_order = []
    for pr in AG_PAIRS:
        for g in range(2):
            for h in pr:
                base = g * 1024 + h * 128
                row_order.extend(range(base, base + 128))
    row_order = np.asarray(row_order)

    in_maps = []
    for c in range(N_CORES):
        b, g = divmod(c, 2)
        cs = slice(g * 1024, (g + 1) * 1024)
        wq = w_attn[:, 0:C][:, cs]
        wk = w_attn[:, C : 2 * C][:, cs]
        wv = w_attn[:, 2 * C : 3 * C][:, cs]
        bq = b_attn[0:C][cs]
        bk = b_attn[C : 2 * C][cs]
        bvv = b_attn[2 * C : 3 * C][cs]
        in_maps.append(
            {
                "xT": np.ascontiguousarray(x[b].T).astype(BF16),
                "wqkv": np.concatenate([wq, wk, wv], axis=1).astype(BF16),
                "bqk": np.ascontiguousarray(
                    np.concatenate([bq, bk]).reshape(16, 128).T
                ).astype(np.float32),
                "bv": bvv.reshape(1, 1024).astype(np.float32),
                "cosT": cosT,
                "sinT": sinT,
                "maskT": maskT,
                "perm": permM,
                "ident": identM,
                "ind2": ind2,
                "sel2": sel2,
                "wproj": w_proj[row_order][:, cs].astype(BF16),
                "bproj": b_proj[cs].reshape(1, 1024).astype(np.float32),
            }
        )
    return in_maps


def kernel(x, w_attn, b_attn, w_proj, b_proj, _trace=False):
    from concourse.bass_utils import run_bass_kernel_spmd

    if "nc" not in _PROGRAM_CACHE:
        _PROGRAM_CACHE["nc"] = _build_program()
    nc = _PROGRAM_CACHE["nc"]

    in_maps = _host_inputs(x, w_attn, b_attn, w_proj, b_proj)
    res = run_bass_kernel_spmd(
        nc, in_maps, core_ids=list(range(N_CORES)), trace=_trace
    )
    _PROGRAM_CACHE["last_results"] = res

    out = np.zeros((B, T, C), np.float32)
    for c in range(N_CORES):
        b, g = divmod(c, 2)
        out[b, :, g * 1024 : (g + 1) * 1024] = res.results[c]["out"]
    return out
